# revision 20
# baseline (speedup 1.0000x reference)
"""Compact bilinear pooling (count-sketch + FFT) Trainium2 kernel.

Math: for each image, y = irfft( sum_over_pixels( rfft(x_px @ S1) * rfft(x_px @ S2) ) ),
then signed-sqrt and L2 normalization.  Since rfft(x @ S) == x @ rfft(S), the
per-pixel FFTs become plain matmuls against W = rfft(S, axis=1) (precomputed on
host once per call), and since the inverse FFT is linear it is applied AFTER
spatial sum pooling, so only 4 rows per device need the inverse transform. The
inverse rfft of the pooled spectrum is computed on-device as a factored
(Cooley-Tukey, 128x64) real IDFT using two small matmul/twiddle stages.

Sharding: data-parallel over 8 NeuronCores, 4 images each; W / DFT bases are
replicated.  Everything except the rfft(S) weight prep runs on device.

Perf notes (v2): all PE matmuls in bf16 (fp32r is same-rate but doubles DMA);
W streamed as pre-tiled contiguous 256KB blocks; the complex product + spatial
pooling is split across DVE (2 muls + fused 2-image reduces) and Pool (2 muls
+ combine adds) reading PSUM directly, so no PSUM->SBUF staging copies; the
nyquist bin gets a skinny 1-row matmul instead of a full 128-row tile; the
signed-sqrt/L2 epilogue runs on a [128,256] reshape of the [4,8192] result so
all 128 lanes are used.
"""

import os
import numpy as np

import concourse.bass as bass
import concourse.bacc as bacc
import concourse.mybir as mybir
import concourse.tile as tile
from concourse.bass_utils import run_bass_kernel_spmd

D = 8192          # projection dim
CH = 512          # input channels
HW = 196          # pixels per image (14x14)
B = 32            # batch
NCORES = 8
BPD = B // NCORES     # images per device (4)
RWS = BPD * HW        # spatial rows per device (784)
NTM = 32              # main freq tiles of 128 (4096 freqs); nyquist separate
F32 = mybir.dt.float32
BF = mybir.dt.bfloat16
NPBF = mybir.dt.np(BF)

AX = mybir.AxisListType
ALU = mybir.AluOpType
ACT = mybir.ActivationFunctionType


def _build():
    nc = bacc.Bacc(None, target_bir_lowering=False)

    xd = nc.dram_tensor("xdev", [128, 4, RWS], BF, kind="ExternalInput")
    wtd = nc.dram_tensor("wtd", [NTM, 128, 4, 4, 128], BF, kind="ExternalInput")
    wnd = nc.dram_tensor("wnd", [128, 4, 2], BF, kind="ExternalInput")
    cbt = nc.dram_tensor("cbt", [128, D], BF, kind="ExternalInput")
    sbt = nc.dram_tensor("sbt", [128, D], BF, kind="ExternalInput")
    cwf = nc.dram_tensor("cwf", [128, 512], F32, kind="ExternalInput")
    swf = nc.dram_tensor("swf", [128, 512], F32, kind="ExternalInput")
    selr = nc.dram_tensor("selr", [128, 4], BF, kind="ExternalInput")
    altr = nc.dram_tensor("altr", [1, 512], BF, kind="ExternalInput")
    selg = nc.dram_tensor("selg", [128, 4], F32, kind="ExternalInput")
    selgT = nc.dram_tensor("selgT", [4, 128], F32, kind="ExternalInput")
    yd = nc.dram_tensor("ydev", [BPD, D], F32, kind="ExternalOutput")
    ybd = nc.dram_tensor("ybounce", [BPD, D], F32, kind="Internal")

    with tile.TileContext(nc) as tc:
        with tc.tile_pool(name="singles", bufs=1) as singles:
            x_sb = singles.tile([128, 4, RWS], BF)
            nc.sync.dma_start(out=x_sb, in_=xd[:, :, :])

            # nyquist weights on the sync queue (needed in the first few us;
            # the gpsimd SWDGE queue starts ~15us late behind pool DRAINs)
            wn_sb = singles.tile([128, 4, 2], BF)
            nc.sync.dma_start(out=wn_sb, in_=wnd[:, :, :])
            # remaining constants are only needed in phase B; the gpsimd
            # (SWDGE) path keeps them off the sync-ring W-tile stream.
            cb_sb = singles.tile([128, D], BF)
            nc.gpsimd.dma_start(out=cb_sb, in_=cbt[:, :])
            sb_sb = singles.tile([128, D], BF)
            nc.gpsimd.dma_start(out=sb_sb, in_=sbt[:, :])
            cwf_sb = singles.tile([128, 512], F32)
            nc.gpsimd.dma_start(out=cwf_sb, in_=cwf[:, :])
            swf_sb = singles.tile([128, 512], F32)
            nc.gpsimd.dma_start(out=swf_sb, in_=swf[:, :])
            selr_sb = singles.tile([128, 4], BF)
            nc.gpsimd.dma_start(out=selr_sb, in_=selr[:, :])
            alt_sb = singles.tile([1, 512], BF)
            nc.gpsimd.dma_start(out=alt_sb, in_=altr[:, :])
            selg_sb = singles.tile([128, 4], F32)
            nc.gpsimd.dma_start(out=selg_sb, in_=selg[:, :])
            selgT_sb = singles.tile([4, 128], F32)
            nc.gpsimd.dma_start(out=selgT_sb, in_=selgT[:, :])

            # pooled half-spectrum, [k1=128, 4*t + img]; cols 128..131 hold the
            # nyquist products (row 0 only is meaningful there)
            P_r = singles.tile([128, 132], F32)
            P_i = singles.tile([128, 132], F32)

            # ---------------- nyquist bin (k=4096): real-only, 1-row matmuls
            with tc.tile_pool(name="nyp", bufs=1, space="PSUM") as nyp, \
                 tc.tile_pool(name="nys", bufs=1) as nys:
                for h in range(2):
                    rsl = slice(h * 392, (h + 1) * 392)
                    fn1 = nyp.tile([1, 392], F32, tag="fn1")
                    fn2 = nyp.tile([1, 392], F32, tag="fn2")
                    for cc in range(4):
                        nc.tensor.matmul(fn1, lhsT=wn_sb[:, cc, 0:1],
                                         rhs=x_sb[:, cc, rsl],
                                         start=(cc == 0), stop=(cc == 3))
                    for cc in range(4):
                        nc.tensor.matmul(fn2, lhsT=wn_sb[:, cc, 1:2],
                                         rhs=x_sb[:, cc, rsl],
                                         start=(cc == 0), stop=(cc == 3))
                    fn2c = nys.tile([1, 392], BF, tag="fn2c")
                    nc.scalar.copy(fn2c, fn2)
                    mn = nys.tile([1, 392], F32, tag="mn")
                    nc.vector.tensor_mul(mn, fn1, fn2c)
                    nc.vector.reduce_sum(
                        P_r[0:1, 128 + 2 * h:130 + 2 * h],
                        mn[0:1, :].rearrange("p (i r) -> p i r", i=2),
                        axis=AX.X)

            # ---------------- phase A: projections + pooled spectral products
            with tc.tile_pool(name="wp", bufs=3) as wp, \
                 tc.tile_pool(name="fp", bufs=2, space="PSUM") as fp, \
                 tc.tile_pool(name="mp", bufs=3) as mp:
                for t in range(NTM):
                    w_sb = wp.tile([128, 4, 4, 128], BF, tag="w")
                    nc.sync.dma_start(out=w_sb, in_=wtd[t])
                    for h in range(2):
                        rsl = slice(h * 392, (h + 1) * 392)
                        # paired PSUM tiles (512-wide slots for bank alignment):
                        # f12 = [a; b] (re/im of f1), f34 = [c; d] (re/im of f2)
                        f12 = fp.tile([128, 2, 512], F32, tag="f12", name="f12")
                        f34 = fp.tile([128, 2, 512], F32, tag="f34", name="f34")
                        # Act stages each m-matrix to SBUF bf16 right after its
                        # 4 matmuls (DVE has a single PSUM read port and 2x
                        # packing needs SBUF); per-m copies free PSUM earlier.
                        ab16 = mp.tile([128, 2, 392], BF, tag="ab")
                        cd16 = mp.tile([128, 2, 392], BF, tag="cd")
                        for m in range(4):
                            dst = (f12 if m < 2 else f34)[:, m % 2, 0:392]
                            for cc in range(4):
                                nc.tensor.matmul(
                                    dst,
                                    lhsT=w_sb[:, m, cc, :],
                                    rhs=x_sb[:, cc, rsl],
                                    start=(cc == 0),
                                    stop=(cc == 3),
                                )
                            cp16 = (ab16 if m < 2 else cd16)[:, m % 2, :]
                            nc.scalar.copy(cp16, dst)
                        # complex product f1*f2, pooled per image: DVE runs
                        # two paired muls at 2x + both fused 2-image reduces;
                        # Pool does the two bf16 combines.
                        # mA = [a*c; b*c], mB = [a*d; b*d] (c/d broadcast)
                        mA = mp.tile([128, 2, 392], BF, tag="mA")
                        nc.vector.tensor_mul(
                            mA, ab16,
                            cd16[:, 0:1, :].broadcast_to((128, 2, 392)))
                        mB = mp.tile([128, 2, 392], BF, tag="mB")
                        nc.vector.tensor_mul(
                            mB, ab16,
                            cd16[:, 1:2, :].broadcast_to((128, 2, 392)))
                        d_r = mp.tile([128, 392], BF, tag="dr")
                        nc.gpsimd.tensor_sub(d_r, mA[:, 0, :], mB[:, 1, :])
                        d_i = mp.tile([128, 392], BF, tag="di")
                        nc.gpsimd.tensor_add(d_i, mB[:, 0, :], mA[:, 1, :])
                        c0 = 4 * t + 2 * h
                        nc.vector.reduce_sum(
                            P_r[:, c0:c0 + 2],
                            d_r[:, :].rearrange("p (i r) -> p i r", i=2),
                            axis=AX.X)
                        nc.vector.reduce_sum(
                            P_i[:, c0:c0 + 2],
                            d_i[:, :].rearrange("p (i r) -> p i r", i=2),
                            axis=AX.X)

            # ---------------- phase B: factored inverse rfft of pooled spectrum
            # DC bin: bases carry 2/D, k=0 needs 1/D
            nc.vector.tensor_scalar_mul(P_r[0:1, 0:4], P_r[0:1, 0:4], 0.5)
            qr = singles.tile([128, 128], BF)
            nc.vector.tensor_copy(qr, P_r[:, 0:128])
            qi = singles.tile([128, 128], BF)
            nc.vector.tensor_scalar_mul(qi, P_i[:, 0:128], -1.0)
            qrn = singles.tile([128, 128], BF)
            nc.vector.tensor_scalar_mul(qrn, P_r[:, 0:128], -1.0)
            pnyq = singles.tile([1, 4], BF)
            nc.vector.tensor_copy(pnyq, P_r[0:1, 128:132])

            ycat = singles.tile([4, D], F32)
            with tc.tile_pool(name="abp", bufs=2, space="PSUM") as abp, \
                 tc.tile_pool(name="zp", bufs=3) as zp, \
                 tc.tile_pool(name="yp", bufs=2, space="PSUM") as yp:
                for chk in range(16):
                    ks = slice(chk * 512, (chk + 1) * 512)
                    a_ps = abp.tile([128, 512], F32, tag="a")
                    b_ps = abp.tile([128, 512], F32, tag="b")
                    nc.tensor.matmul(a_ps, lhsT=qr, rhs=cb_sb[:, ks], start=True, stop=False)
                    nc.tensor.matmul(a_ps, lhsT=qi, rhs=sb_sb[:, ks], start=False, stop=True)
                    nc.tensor.matmul(b_ps, lhsT=qi, rhs=cb_sb[:, ks], start=True, stop=False)
                    nc.tensor.matmul(b_ps, lhsT=qrn, rhs=sb_sb[:, ks], start=False, stop=True)
                    z1 = zp.tile([128, 512], F32, tag="z1")
                    nc.vector.tensor_mul(z1, a_ps, cwf_sb)
                    z2 = zp.tile([128, 512], F32, tag="z2")
                    nc.vector.tensor_mul(z2, b_ps, swf_sb)
                    z = zp.tile([128, 512], BF, tag="z")
                    nc.gpsimd.tensor_add(z, z1, z2)
                    y4 = yp.tile([4, 512], F32, tag="y4")
                    nc.tensor.matmul(y4, lhsT=selr_sb, rhs=z, start=True, stop=False)
                    nc.tensor.matmul(y4, lhsT=pnyq, rhs=alt_sb, start=False, stop=True)
                    nc.scalar.copy(ycat[:, ks], y4)

            # ------------ epilogue: signed sqrt + L2 normalize, on a
            # [128, 256] reshape of ycat so all 128 lanes are used.
            with tc.tile_pool(name="ep", bufs=1) as ep, \
                 tc.tile_pool(name="epp", bufs=1, space="PSUM") as epp:
                # bounce [4, 8192] through DRAM to reshape to [128, 256]
                # (cross-partition SBUF rearrange is not a valid AP)
                nc.sync.dma_start(out=ybd[:, :], in_=ycat)
                yw = ep.tile([128, 256], F32)
                nc.sync.dma_start(
                    out=yw, in_=ybd.rearrange("i (g c) -> (i g) c", g=32))
                aw = ep.tile([128, 256], F32)
                rsum = ep.tile([128, 1], F32)
                # |y| and its per-partition sum in one activation pass
                nc.scalar.activation(aw, yw, ACT.Abs, accum_out=rsum)
                # per-image sum over the 32 partitions of each image group
                ns_ps = epp.tile([4, 1], F32)
                nc.tensor.matmul(ns_ps, lhsT=selg_sb, rhs=rsum,
                                 start=True, stop=True)
                # ||y_ss||^2 = sum(|y| + 1e-8) = sum|y| + D*1e-8
                e2 = ep.tile([4, 1], F32)
                nc.vector.memset(e2, float(D * 1e-8))
                nrm = ep.tile([4, 1], F32)
                nc.scalar.activation(nrm, ns_ps, ACT.Sqrt, bias=e2)
                inv = ep.tile([4, 1], F32)
                nc.vector.reciprocal(inv, nrm)
                # broadcast inv back to the 128 partition rows
                ib_ps = epp.tile([128, 1], F32)
                nc.tensor.matmul(ib_ps, lhsT=selgT_sb, rhs=inv,
                                 start=True, stop=True)
                invb = ep.tile([128, 1], F32)
                nc.scalar.copy(invb, ib_ps)
                # ss = sqrt(|y| + 1e-8), in place over aw
                e1 = ep.tile([128, 1], F32)
                nc.vector.memset(e1, 1e-8)
                nc.scalar.activation(aw, aw, ACT.Sqrt, bias=e1)
                sg = ep.tile([128, 256], F32)
                nc.scalar.activation(sg, yw, ACT.Sign)
                nc.vector.tensor_mul(aw, aw, sg)
                nc.vector.tensor_scalar_mul(sg, aw, invb)
                nc.sync.dma_start(
                    out=yd.rearrange("i (g c) -> (i g) c", g=32), in_=sg)
    return nc


_CACHE = {}


def _enable_axon_tracing():
    """Best-effort NTFF profiling shims for the axon agent image (test-only)."""
    if _CACHE.get("trace_shimmed"):
        return
    import sys
    import types
    try:
        from antenv.axon_hooks import get_axon_ntff_profile_hook  # noqa: F401
    except ImportError:
        try:
            from trn_agent_boot.trn_boot import _ntff_profile_via_ctypes
            hook = _ntff_profile_via_ctypes("/opt/axon/libaxon_pjrt.so")
            m = types.ModuleType("antenv.axon_hooks")
            m.get_axon_ntff_profile_hook = lambda: hook
            m.set_axon_ntff_profile_hook = lambda h: None
            sys.modules["antenv.axon_hooks"] = m
        except Exception as e:  # pragma: no cover
            print("tracing shim unavailable:", e)
    try:
        import concourse.bass_utils as bu
        bu.upload_artifacts = lambda tmpdir: f"local://{tmpdir}"
    except Exception as e:  # pragma: no cover
        print("upload shim failed:", e)
    _CACHE["trace_shimmed"] = True


def _host_consts():
    if "consts" in _CACHE:
        return _CACHE["consts"]
    k1 = np.arange(128, dtype=np.int64)[:, None]
    n = np.arange(D, dtype=np.int64)[None, :]
    ang = 2.0 * np.pi * ((k1 * n) % D) / D
    cbt = (np.cos(ang) * (2.0 / D)).astype(NPBF)
    sbt = (np.sin(ang) * (2.0 / D)).astype(NPBF)

    p = np.arange(128, dtype=np.int64)[:, None]
    j = np.arange(512, dtype=np.int64)[None, :]
    ang2 = 2.0 * np.pi * ((p // 4) * (j % 64) % 64) / 64.0
    cwf = np.cos(ang2).astype(np.float32)
    swf = np.sin(ang2).astype(np.float32)

    sel = np.zeros((128, 4), np.float32)
    sel[np.arange(128), np.arange(128) % 4] = 1.0
    alt = (((-1.0) ** np.arange(512)) / D).astype(NPBF)[None, :]
    # image-group (p // 32) selection matrices for the epilogue norm
    sg_ = np.zeros((128, 4), np.float32)
    sg_[np.arange(128), np.arange(128) // 32] = 1.0
    _CACHE["consts"] = (cbt, sbt, cwf, swf, sel.astype(NPBF), alt,
                        sg_, np.ascontiguousarray(sg_.T))
    return _CACHE["consts"]


def kernel(x, S1, S2):
    x = np.ascontiguousarray(x, dtype=np.float32)
    S1 = np.asarray(S1, dtype=np.float32)
    S2 = np.asarray(S2, dtype=np.float32)

    W1 = np.fft.rfft(S1.astype(np.float64), axis=1)  # [512, 4097]
    W2 = np.fft.rfft(S2.astype(np.float64), axis=1)
    # pre-tiled weights: wtd[t, p, m, cc, k] = Wm[cc*128+p, 128*t+k]
    warr = np.stack([W1.real[:, :4096], W1.imag[:, :4096],
                     W2.real[:, :4096], W2.imag[:, :4096]], axis=0)
    wtd = np.ascontiguousarray(
        warr.reshape(4, 4, 128, NTM, 128).transpose(3, 2, 0, 1, 4)
    ).astype(NPBF)
    # nyquist column (k=4096, purely real): wnd[p, cc, m2]
    wnd = np.ascontiguousarray(
        np.stack([W1.real[:, 4096], W2.real[:, 4096]], axis=1)
        .reshape(4, 128, 2).transpose(1, 0, 2)
    ).astype(NPBF)

    cbt, sbt, cwf, swf, sel, alt, sg_, sgT = _host_consts()

    if "nc" not in _CACHE:
        nc = _build()
        nc.finalize()
        _CACHE["nc"] = nc
    nc = _CACHE["nc"]

    common = {
        "wtd": wtd, "wnd": wnd, "cbt": cbt, "sbt": sbt, "cwf": cwf,
        "swf": swf, "selr": sel, "altr": alt, "selg": sg_, "selgT": sgT,
    }
    in_maps = []
    for d in range(NCORES):
        xdev = np.ascontiguousarray(
            x[d * BPD:(d + 1) * BPD].transpose(1, 0, 2, 3).reshape(CH, RWS)
            .reshape(4, 128, RWS).transpose(1, 0, 2)
        ).astype(NPBF)
        in_maps.append({"xdev": xdev, **common})

    trace = bool(int(os.environ.get("CBP_TRACE", "0")))
    if trace:
        _enable_axon_tracing()
    res = run_bass_kernel_spmd(nc, in_maps, list(range(NCORES)), trace=trace)
    _CACHE["last_results"] = res
    out = np.concatenate(
        [np.asarray(res.results[d]["ydev"]) for d in range(NCORES)], axis=0
    )
    return out.astype(np.float32)


# revision 22
# speedup vs baseline: 1.0058x; 1.0058x over previous
"""Compact bilinear pooling (count-sketch + FFT) Trainium2 kernel.

Math: for each image, y = irfft( sum_over_pixels( rfft(x_px @ S1) * rfft(x_px @ S2) ) ),
then signed-sqrt and L2 normalization.  Since rfft(x @ S) == x @ rfft(S), the
per-pixel FFTs become plain matmuls against W = rfft(S, axis=1) (precomputed on
host once per call), and since the inverse FFT is linear it is applied AFTER
spatial sum pooling, so only 4 rows per device need the inverse transform. The
inverse rfft of the pooled spectrum is computed on-device as a factored
(Cooley-Tukey, 128x64) real IDFT using two small matmul/twiddle stages.

Sharding: data-parallel over 8 NeuronCores, 4 images each; W / DFT bases are
replicated.  Everything except the rfft(S) weight prep runs on device.

Perf notes (v2): all PE matmuls in bf16 (fp32r is same-rate but doubles DMA);
W streamed as pre-tiled contiguous 256KB blocks; the complex product + spatial
pooling is split across DVE (2 muls + fused 2-image reduces) and Pool (2 muls
+ combine adds) reading PSUM directly, so no PSUM->SBUF staging copies; the
nyquist bin gets a skinny 1-row matmul instead of a full 128-row tile; the
signed-sqrt/L2 epilogue runs on a [128,256] reshape of the [4,8192] result so
all 128 lanes are used.
"""

import os
import numpy as np

import concourse.bass as bass
import concourse.bacc as bacc
import concourse.mybir as mybir
import concourse.tile as tile
from concourse.bass_utils import run_bass_kernel_spmd

D = 8192          # projection dim
CH = 512          # input channels
HW = 196          # pixels per image (14x14)
B = 32            # batch
NCORES = 8
BPD = B // NCORES     # images per device (4)
RWS = BPD * HW        # spatial rows per device (784)
NTM = 32              # main freq tiles of 128 (4096 freqs); nyquist separate
F32 = mybir.dt.float32
BF = mybir.dt.bfloat16
NPBF = mybir.dt.np(BF)

AX = mybir.AxisListType
ALU = mybir.AluOpType
ACT = mybir.ActivationFunctionType


def _build():
    nc = bacc.Bacc(None, target_bir_lowering=False)

    xd = nc.dram_tensor("xdev", [128, 4, RWS], BF, kind="ExternalInput")
    wtd = nc.dram_tensor("wtd", [NTM, 128, 4, 4, 128], BF, kind="ExternalInput")
    wnd = nc.dram_tensor("wnd", [128, 4, 2], BF, kind="ExternalInput")
    cbt = nc.dram_tensor("cbt", [128, D], BF, kind="ExternalInput")
    sbt = nc.dram_tensor("sbt", [128, D], BF, kind="ExternalInput")
    cwf = nc.dram_tensor("cwf", [128, 512], F32, kind="ExternalInput")
    swf = nc.dram_tensor("swf", [128, 512], F32, kind="ExternalInput")
    selr = nc.dram_tensor("selr", [128, 4], BF, kind="ExternalInput")
    altr = nc.dram_tensor("altr", [1, 512], BF, kind="ExternalInput")
    selg = nc.dram_tensor("selg", [128, 4], F32, kind="ExternalInput")
    selgT = nc.dram_tensor("selgT", [4, 128], F32, kind="ExternalInput")
    yd = nc.dram_tensor("ydev", [BPD, D], F32, kind="ExternalOutput")
    ybd = nc.dram_tensor("ybounce", [BPD, D], F32, kind="Internal")

    with tile.TileContext(nc) as tc:
        with tc.tile_pool(name="singles", bufs=1) as singles:
            x_sb = singles.tile([128, 4, RWS], BF)
            nc.sync.dma_start(out=x_sb, in_=xd[:, :, :])

            # nyquist weights on the sync queue (needed in the first few us;
            # the gpsimd SWDGE queue starts ~15us late behind pool DRAINs)
            wn_sb = singles.tile([128, 4, 2], BF)
            nc.sync.dma_start(out=wn_sb, in_=wnd[:, :, :])
            # remaining constants are only needed in phase B; allocate now but
            # DMA them mid-loop (gpsimd/SWDGE queue) so they don't contend
            # for HBM bandwidth during the startup-critical x/wn/W0 loads.
            cb_sb = singles.tile([128, D], BF)
            sb_sb = singles.tile([128, D], BF)
            cwf_sb = singles.tile([128, 512], F32)
            swf_sb = singles.tile([128, 512], F32)
            selr_sb = singles.tile([128, 4], BF)
            alt_sb = singles.tile([1, 512], BF)
            selg_sb = singles.tile([128, 4], F32)
            selgT_sb = singles.tile([4, 128], F32)

            # pooled half-spectrum, [k1=128, 4*t + img]; cols 128..131 hold the
            # nyquist products (row 0 only is meaningful there)
            P_r = singles.tile([128, 132], F32)
            P_i = singles.tile([128, 132], F32)

            # ---------------- nyquist bin (k=4096): real-only, 1-row matmuls
            with tc.tile_pool(name="nyp", bufs=1, space="PSUM") as nyp, \
                 tc.tile_pool(name="nys", bufs=1) as nys:
                for h in range(2):
                    rsl = slice(h * 392, (h + 1) * 392)
                    fn1 = nyp.tile([1, 392], F32, tag="fn1")
                    fn2 = nyp.tile([1, 392], F32, tag="fn2")
                    for cc in range(4):
                        nc.tensor.matmul(fn1, lhsT=wn_sb[:, cc, 0:1],
                                         rhs=x_sb[:, cc, rsl],
                                         start=(cc == 0), stop=(cc == 3))
                    for cc in range(4):
                        nc.tensor.matmul(fn2, lhsT=wn_sb[:, cc, 1:2],
                                         rhs=x_sb[:, cc, rsl],
                                         start=(cc == 0), stop=(cc == 3))
                    fn2c = nys.tile([1, 392], BF, tag="fn2c")
                    nc.scalar.copy(fn2c, fn2)
                    mn = nys.tile([1, 392], F32, tag="mn")
                    nc.vector.tensor_mul(mn, fn1, fn2c)
                    nc.vector.reduce_sum(
                        P_r[0:1, 128 + 2 * h:130 + 2 * h],
                        mn[0:1, :].rearrange("p (i r) -> p i r", i=2),
                        axis=AX.X)

            # ---------------- phase A: projections + pooled spectral products
            with tc.tile_pool(name="wp", bufs=3) as wp, \
                 tc.tile_pool(name="fp", bufs=2, space="PSUM") as fp, \
                 tc.tile_pool(name="mp", bufs=3) as mp:
                for t in range(NTM):
                    w_sb = wp.tile([128, 4, 4, 128], BF, tag=f"w{t % 3}",
                                   bufs=1)
                    nc.sync.dma_start(out=w_sb, in_=wtd[t])
                    if t == 2:
                        # phase-B constants, now that startup DMAs are done
                        nc.gpsimd.dma_start(out=cb_sb, in_=cbt[:, :])
                        nc.gpsimd.dma_start(out=sb_sb, in_=sbt[:, :])
                        nc.gpsimd.dma_start(out=cwf_sb, in_=cwf[:, :])
                        nc.gpsimd.dma_start(out=swf_sb, in_=swf[:, :])
                        nc.gpsimd.dma_start(out=selr_sb, in_=selr[:, :])
                        nc.gpsimd.dma_start(out=alt_sb, in_=altr[:, :])
                        nc.gpsimd.dma_start(out=selg_sb, in_=selg[:, :])
                        nc.gpsimd.dma_start(out=selgT_sb, in_=selgT[:, :])
                    for h in range(2):
                        rsl = slice(h * 392, (h + 1) * 392)
                        # paired PSUM tiles (512-wide slots for bank alignment):
                        # f12 = [a; b] (re/im of f1), f34 = [c; d] (re/im of f2)
                        f12 = fp.tile([128, 2, 512], F32, tag="f12", name="f12")
                        f34 = fp.tile([128, 2, 512], F32, tag="f34", name="f34")
                        # Act stages each m-matrix to SBUF bf16 right after its
                        # 4 matmuls (DVE has a single PSUM read port and 2x
                        # packing needs SBUF); per-m copies free PSUM earlier.
                        ab16 = mp.tile([128, 2, 392], BF, tag="ab")
                        cd16 = mp.tile([128, 2, 392], BF, tag="cd")
                        for m in range(4):
                            dst = (f12 if m < 2 else f34)[:, m % 2, 0:392]
                            for cc in range(4):
                                nc.tensor.matmul(
                                    dst,
                                    lhsT=w_sb[:, m, cc, :],
                                    rhs=x_sb[:, cc, rsl],
                                    start=(cc == 0),
                                    stop=(cc == 3),
                                )
                            cp16 = (ab16 if m < 2 else cd16)[:, m % 2, :]
                            nc.scalar.copy(cp16, dst)
                        # complex product f1*f2, pooled per image: DVE runs
                        # two paired muls at 2x + both fused 2-image reduces;
                        # Pool does the two bf16 combines.
                        # mA = [a*c; b*c], mB = [a*d; b*d] (c/d broadcast)
                        mA = mp.tile([128, 2, 392], BF, tag="mA")
                        nc.vector.tensor_mul(
                            mA, ab16,
                            cd16[:, 0:1, :].broadcast_to((128, 2, 392)))
                        mB = mp.tile([128, 2, 392], BF, tag="mB")
                        nc.vector.tensor_mul(
                            mB, ab16,
                            cd16[:, 1:2, :].broadcast_to((128, 2, 392)))
                        d_r = mp.tile([128, 392], BF, tag="dr")
                        nc.gpsimd.tensor_sub(d_r, mA[:, 0, :], mB[:, 1, :])
                        d_i = mp.tile([128, 392], BF, tag="di")
                        nc.gpsimd.tensor_add(d_i, mB[:, 0, :], mA[:, 1, :])
                        c0 = 4 * t + 2 * h
                        nc.vector.reduce_sum(
                            P_r[:, c0:c0 + 2],
                            d_r[:, :].rearrange("p (i r) -> p i r", i=2),
                            axis=AX.X)
                        nc.vector.reduce_sum(
                            P_i[:, c0:c0 + 2],
                            d_i[:, :].rearrange("p (i r) -> p i r", i=2),
                            axis=AX.X)

            # ---------------- phase B: factored inverse rfft of pooled spectrum
            # DC bin: bases carry 2/D, k=0 needs 1/D
            nc.vector.tensor_scalar_mul(P_r[0:1, 0:4], P_r[0:1, 0:4], 0.5)
            qr = singles.tile([128, 128], BF)
            nc.vector.tensor_copy(qr, P_r[:, 0:128])
            qi = singles.tile([128, 128], BF)
            nc.vector.tensor_scalar_mul(qi, P_i[:, 0:128], -1.0)
            qrn = singles.tile([128, 128], BF)
            nc.vector.tensor_scalar_mul(qrn, P_r[:, 0:128], -1.0)
            pnyq = singles.tile([1, 4], BF)
            nc.vector.tensor_copy(pnyq, P_r[0:1, 128:132])

            ycat = singles.tile([4, D], F32)
            with tc.tile_pool(name="abp", bufs=2, space="PSUM") as abp, \
                 tc.tile_pool(name="zp", bufs=3) as zp, \
                 tc.tile_pool(name="yp", bufs=2, space="PSUM") as yp:
                for chk in range(16):
                    ks = slice(chk * 512, (chk + 1) * 512)
                    a_ps = abp.tile([128, 512], F32, tag="a")
                    b_ps = abp.tile([128, 512], F32, tag="b")
                    nc.tensor.matmul(a_ps, lhsT=qr, rhs=cb_sb[:, ks], start=True, stop=False)
                    nc.tensor.matmul(a_ps, lhsT=qi, rhs=sb_sb[:, ks], start=False, stop=True)
                    nc.tensor.matmul(b_ps, lhsT=qi, rhs=cb_sb[:, ks], start=True, stop=False)
                    nc.tensor.matmul(b_ps, lhsT=qrn, rhs=sb_sb[:, ks], start=False, stop=True)
                    z1 = zp.tile([128, 512], F32, tag="z1")
                    nc.vector.tensor_mul(z1, a_ps, cwf_sb)
                    z2 = zp.tile([128, 512], F32, tag="z2")
                    nc.vector.tensor_mul(z2, b_ps, swf_sb)
                    z = zp.tile([128, 512], BF, tag="z")
                    nc.gpsimd.tensor_add(z, z1, z2)
                    y4 = yp.tile([4, 512], F32, tag="y4")
                    nc.tensor.matmul(y4, lhsT=selr_sb, rhs=z, start=True, stop=False)
                    nc.tensor.matmul(y4, lhsT=pnyq, rhs=alt_sb, start=False, stop=True)
                    nc.scalar.copy(ycat[:, ks], y4)

            # ------------ epilogue: signed sqrt + L2 normalize, on a
            # [128, 256] reshape of ycat so all 128 lanes are used.
            with tc.tile_pool(name="ep", bufs=1) as ep, \
                 tc.tile_pool(name="epp", bufs=1, space="PSUM") as epp:
                # bounce [4, 8192] through DRAM to reshape to [128, 256]
                # (cross-partition SBUF rearrange is not a valid AP)
                nc.sync.dma_start(out=ybd[:, :], in_=ycat)
                yw = ep.tile([128, 256], F32)
                nc.sync.dma_start(
                    out=yw, in_=ybd.rearrange("i (g c) -> (i g) c", g=32))
                aw = ep.tile([128, 256], F32)
                rsum = ep.tile([128, 1], F32)
                # |y| and its per-partition sum in one activation pass
                nc.scalar.activation(aw, yw, ACT.Abs, accum_out=rsum)
                # per-image sum over the 32 partitions of each image group
                ns_ps = epp.tile([4, 1], F32)
                nc.tensor.matmul(ns_ps, lhsT=selg_sb, rhs=rsum,
                                 start=True, stop=True)
                # ||y_ss||^2 = sum(|y| + 1e-8) = sum|y| + D*1e-8
                e2 = ep.tile([4, 1], F32)
                nc.vector.memset(e2, float(D * 1e-8))
                nrm = ep.tile([4, 1], F32)
                nc.scalar.activation(nrm, ns_ps, ACT.Sqrt, bias=e2)
                inv = ep.tile([4, 1], F32)
                nc.vector.reciprocal(inv, nrm)
                # broadcast inv back to the 128 partition rows
                ib_ps = epp.tile([128, 1], F32)
                nc.tensor.matmul(ib_ps, lhsT=selgT_sb, rhs=inv,
                                 start=True, stop=True)
                invb = ep.tile([128, 1], F32)
                nc.scalar.copy(invb, ib_ps)
                # ss = sqrt(|y| + 1e-8), in place over aw
                e1 = ep.tile([128, 1], F32)
                nc.vector.memset(e1, 1e-8)
                nc.scalar.activation(aw, aw, ACT.Sqrt, bias=e1)
                sg = ep.tile([128, 256], F32)
                nc.scalar.activation(sg, yw, ACT.Sign)
                nc.vector.tensor_mul(aw, aw, sg)
                nc.vector.tensor_scalar_mul(sg, aw, invb)
                nc.sync.dma_start(
                    out=yd.rearrange("i (g c) -> (i g) c", g=32), in_=sg)
    return nc


_CACHE = {}


def _enable_axon_tracing():
    """Best-effort NTFF profiling shims for the axon agent image (test-only)."""
    if _CACHE.get("trace_shimmed"):
        return
    import sys
    import types
    try:
        from antenv.axon_hooks import get_axon_ntff_profile_hook  # noqa: F401
    except ImportError:
        try:
            from trn_agent_boot.trn_boot import _ntff_profile_via_ctypes
            hook = _ntff_profile_via_ctypes("/opt/axon/libaxon_pjrt.so")
            m = types.ModuleType("antenv.axon_hooks")
            m.get_axon_ntff_profile_hook = lambda: hook
            m.set_axon_ntff_profile_hook = lambda h: None
            sys.modules["antenv.axon_hooks"] = m
        except Exception as e:  # pragma: no cover
            print("tracing shim unavailable:", e)
    try:
        import concourse.bass_utils as bu
        bu.upload_artifacts = lambda tmpdir: f"local://{tmpdir}"
    except Exception as e:  # pragma: no cover
        print("upload shim failed:", e)
    _CACHE["trace_shimmed"] = True


def _host_consts():
    if "consts" in _CACHE:
        return _CACHE["consts"]
    k1 = np.arange(128, dtype=np.int64)[:, None]
    n = np.arange(D, dtype=np.int64)[None, :]
    ang = 2.0 * np.pi * ((k1 * n) % D) / D
    cbt = (np.cos(ang) * (2.0 / D)).astype(NPBF)
    sbt = (np.sin(ang) * (2.0 / D)).astype(NPBF)

    p = np.arange(128, dtype=np.int64)[:, None]
    j = np.arange(512, dtype=np.int64)[None, :]
    ang2 = 2.0 * np.pi * ((p // 4) * (j % 64) % 64) / 64.0
    cwf = np.cos(ang2).astype(np.float32)
    swf = np.sin(ang2).astype(np.float32)

    sel = np.zeros((128, 4), np.float32)
    sel[np.arange(128), np.arange(128) % 4] = 1.0
    alt = (((-1.0) ** np.arange(512)) / D).astype(NPBF)[None, :]
    # image-group (p // 32) selection matrices for the epilogue norm
    sg_ = np.zeros((128, 4), np.float32)
    sg_[np.arange(128), np.arange(128) // 32] = 1.0
    _CACHE["consts"] = (cbt, sbt, cwf, swf, sel.astype(NPBF), alt,
                        sg_, np.ascontiguousarray(sg_.T))
    return _CACHE["consts"]


def kernel(x, S1, S2):
    x = np.ascontiguousarray(x, dtype=np.float32)
    S1 = np.asarray(S1, dtype=np.float32)
    S2 = np.asarray(S2, dtype=np.float32)

    W1 = np.fft.rfft(S1.astype(np.float64), axis=1)  # [512, 4097]
    W2 = np.fft.rfft(S2.astype(np.float64), axis=1)
    # pre-tiled weights: wtd[t, p, m, cc, k] = Wm[cc*128+p, 128*t+k]
    warr = np.stack([W1.real[:, :4096], W1.imag[:, :4096],
                     W2.real[:, :4096], W2.imag[:, :4096]], axis=0)
    wtd = np.ascontiguousarray(
        warr.reshape(4, 4, 128, NTM, 128).transpose(3, 2, 0, 1, 4)
    ).astype(NPBF)
    # nyquist column (k=4096, purely real): wnd[p, cc, m2]
    wnd = np.ascontiguousarray(
        np.stack([W1.real[:, 4096], W2.real[:, 4096]], axis=1)
        .reshape(4, 128, 2).transpose(1, 0, 2)
    ).astype(NPBF)

    cbt, sbt, cwf, swf, sel, alt, sg_, sgT = _host_consts()

    if "nc" not in _CACHE:
        nc = _build()
        nc.finalize()
        _CACHE["nc"] = nc
    nc = _CACHE["nc"]

    common = {
        "wtd": wtd, "wnd": wnd, "cbt": cbt, "sbt": sbt, "cwf": cwf,
        "swf": swf, "selr": sel, "altr": alt, "selg": sg_, "selgT": sgT,
    }
    in_maps = []
    for d in range(NCORES):
        xdev = np.ascontiguousarray(
            x[d * BPD:(d + 1) * BPD].transpose(1, 0, 2, 3).reshape(CH, RWS)
            .reshape(4, 128, RWS).transpose(1, 0, 2)
        ).astype(NPBF)
        in_maps.append({"xdev": xdev, **common})

    trace = bool(int(os.environ.get("CBP_TRACE", "0")))
    if trace:
        _enable_axon_tracing()
    res = run_bass_kernel_spmd(nc, in_maps, list(range(NCORES)), trace=trace)
    _CACHE["last_results"] = res
    out = np.concatenate(
        [np.asarray(res.results[d]["ydev"]) for d in range(NCORES)], axis=0
    )
    return out.astype(np.float32)
